# revision 1
# baseline (speedup 1.0000x reference)
"""Distributed causal multi-head attention for Trainium2 (8 NeuronCores).

Reference computes, for x [2, 2048, 1024]:
    qkv = x @ W_qkv + b_qkv ; split into q,k,v heads (16 heads, d_k=64)
    causal softmax attention per head
    out = ctx @ W_o + b_o

Sharding (data + head parallel): core c handles batch b=c//4 and heads
H = [4g..4g+3] with g=c%4.  Each core:
  - computes q^T,k^T ([dk, T] layout, head-pairs packed into 128 partitions)
    and v ([T, dk] natural layout, augmented with a ones column so the
    attention-weights matmul also produces softmax denominators),
  - computes its heads' causal T x T attention to get ctx^T [256, T],
  - AllGathers ctx^T within its 4-core batch group -> [1024, T],
  - computes a disjoint 256-column slice of the output projection.
Host-side: shard prep (transpose/slice/bf16-cast) and a pure concat of the
8 output column-slices.  All FLOPs (matmuls, softmax, reductions) on device.
"""

import numpy as np
import ml_dtypes

import concourse.bass as bass
import concourse.mybir as mybir
import concourse.tile as tile
from concourse import bacc
from concourse import bass_utils

BF16 = mybir.dt.bfloat16
F32 = mybir.dt.float32
AF = mybir.ActivationFunctionType

T = 2048
D = 1024
NH = 16
HPC = 4  # heads per core
DK = 64
NCORES = 8
TQ = 512  # q-chunk (free dim of logits^T tiles)
NQC = T // TQ  # 4
NKT = T // 128  # 16 k-tiles
NDT = D // 128  # 8 d-tiles
NTT = T // 128  # 16 t-tiles
VW = DK + 1  # 65: v columns per head incl. ones column
SCALE = 1.0 / 8.0  # 1/sqrt(DK)

TRACE = False  # set True (with profile shim installed) to capture HW profile
LAST_RESULT = {}

_cache = {}


def _build():
    nc = bacc.Bacc("TRN2", target_bir_lowering=False, debug=False,
                   num_devices=NCORES)

    xt = nc.declare_dram_parameter("xt", [D, T], BF16, False)
    wq = nc.declare_dram_parameter("wq", [D, 256], BF16, False)
    wk = nc.declare_dram_parameter("wk", [D, 256], BF16, False)
    wv = nc.declare_dram_parameter("wv", [D, HPC * VW], BF16, False)
    wo = nc.declare_dram_parameter("wo", [D, 256], BF16, False)
    bq = nc.declare_dram_parameter("bq", [128, 2], F32, False)
    bk = nc.declare_dram_parameter("bk", [128, 2], F32, False)
    bv = nc.declare_dram_parameter("bv", [128, HPC * VW], F32, False)
    bo = nc.declare_dram_parameter("bo", [128, 256], F32, False)
    masks = nc.declare_dram_parameter("masks", [128, 4 * TQ], BF16, False)
    out = nc.declare_dram_parameter("out", [T, 256], F32, True)

    cc_in = [nc.dram_tensor(f"cc_in{h}", [DK, T], BF16) for h in range(HPC)]
    cc_out = [nc.dram_tensor(f"cc_out{h}", [4 * DK, T], BF16) for h in range(HPC)]

    with tile.TileContext(nc) as tc, tc.tile_pool(name="pers", bufs=1) as pers:
        # ---------------- persistent SBUF ----------------
        xt_sb = pers.tile([128, NDT * T], BF16, tag="xt_sb", name="xt_sb")
        wq_sb = pers.tile([128, NDT * 256], BF16, tag="wq_sb", name="wq_sb")
        wk_sb = pers.tile([128, NDT * 256], BF16, tag="wk_sb", name="wk_sb")
        wv_sb = pers.tile([128, NDT * HPC * VW], BF16, tag="wv_sb", name="wv_sb")
        wo_sb = pers.tile([128, NDT * 256], BF16, tag="wo_sb", name="wo_sb")
        bq_sb = pers.tile([128, 2], F32, tag="bq_sb", name="bq_sb")
        bk_sb = pers.tile([128, 2], F32, tag="bk_sb", name="bk_sb")
        bv_sb = pers.tile([128, HPC * VW], F32, tag="bv_sb", name="bv_sb")
        bo_sb = pers.tile([128, 256], F32, tag="bo_sb", name="bo_sb")
        mask_sb = pers.tile([128, 4 * TQ], BF16, tag="mask_sb", name="mask_sb")
        qT_sb = pers.tile([128, 2 * T], BF16, tag="qT_sb", name="qT_sb")
        kT_sb = pers.tile([128, 2 * T], BF16, tag="kT_sb", name="kT_sb")
        v_sb = pers.tile([128, NTT * HPC * VW], BF16, tag="v_sb", name="v_sb")
        ctxg_sb = pers.tile([128, NDT * T], BF16, tag="ctxg_sb", name="ctxg_sb")

        for d in range(NDT):
            nc.sync.dma_start(wq_sb[:, d * 256:(d + 1) * 256],
                              wq[128 * d:128 * (d + 1), :])
            nc.sync.dma_start(wk_sb[:, d * 256:(d + 1) * 256],
                              wk[128 * d:128 * (d + 1), :])
            nc.sync.dma_start(wv_sb[:, d * HPC * VW:(d + 1) * HPC * VW],
                              wv[128 * d:128 * (d + 1), :])
            nc.sync.dma_start(wo_sb[:, d * 256:(d + 1) * 256],
                              wo[128 * d:128 * (d + 1), :])
        for qc in range(NQC):
            for d in range(NDT):
                nc.sync.dma_start(
                    xt_sb[:, d * T + qc * TQ:d * T + (qc + 1) * TQ],
                    xt[128 * d:128 * (d + 1), qc * TQ:(qc + 1) * TQ])
        nc.sync.dma_start(bq_sb[:], bq[:])
        nc.sync.dma_start(bk_sb[:], bk[:])
        nc.sync.dma_start(bv_sb[:], bv[:])
        nc.sync.dma_start(bo_sb[:], bo[:])
        nc.sync.dma_start(mask_sb[:], masks[:])

        with (
            tc.tile_pool(name="pp", space="PSUM", bufs=2) as pp,
            tc.tile_pool(name="sp", space="SBUF", bufs=2) as sp,
        ):
            # ---------------- QKV projections (chunked emitters) ----------
            # round-robin QKV psum tiles over all tags (the attention-phase
            # tags are idle during QKV) for deep buffering
            qkv_tags = ["lgX", "lgX", "lgY", "ctxX", "ctxY"]
            qkv_bufs = {"lgX": 2, "lgY": 1, "ctxX": 1, "ctxY": 1}
            qkv_ctr = [0]

            def qkv_tag():
                t = qkv_tags[qkv_ctr[0] % len(qkv_tags)]
                qkv_ctr[0] += 1
                return t

            def emit_qk(p):
                for qc in range(NQC):
                    _t = qkv_tag()
                    psq = pp.tile([128, TQ], F32, tag=_t, bufs=qkv_bufs[_t],
                                  name=f"psq_{p}_{qc}")
                    for d in range(NDT):
                        nc.tensor.matmul(
                            psq[:],
                            lhsT=wq_sb[:, d * 256 + 128 * p:
                                       d * 256 + 128 * p + 128],
                            rhs=xt_sb[:, d * T + qc * TQ:d * T + (qc + 1) * TQ],
                            start=(d == 0), stop=(d == NDT - 1))
                    nc.vector.tensor_scalar_add(
                        qT_sb[:, p * T + qc * TQ:p * T + (qc + 1) * TQ],
                        psq[:], bq_sb[:, p:p + 1])
                    _t = qkv_tag()
                    psk = pp.tile([128, TQ], F32, tag=_t, bufs=qkv_bufs[_t],
                                  name=f"psk_{p}_{qc}")
                    for d in range(NDT):
                        nc.tensor.matmul(
                            psk[:],
                            lhsT=wk_sb[:, d * 256 + 128 * p:
                                       d * 256 + 128 * p + 128],
                            rhs=xt_sb[:, d * T + qc * TQ:d * T + (qc + 1) * TQ],
                            start=(d == 0), stop=(d == NDT - 1))
                    nc.vector.tensor_scalar_add(
                        kT_sb[:, p * T + qc * TQ:p * T + (qc + 1) * TQ],
                        psk[:], bk_sb[:, p:p + 1])

            def emit_v():
                W = HPC * VW
                for tt in range(NTT):
                    _t = qkv_tag()
                    psv = pp.tile([128, W], F32, tag=_t, bufs=qkv_bufs[_t],
                                  name=f"psv_{tt}")
                    for d in range(NDT):
                        nc.tensor.matmul(
                            psv[:],
                            lhsT=xt_sb[:, d * T + tt * 128:d * T + (tt + 1) * 128],
                            rhs=wv_sb[:, d * W:(d + 1) * W],
                            start=(d == 0), stop=(d == NDT - 1))
                    nc.vector.tensor_add(v_sb[:, tt * W:(tt + 1) * W],
                                         psv[:], bv_sb[:])

            # ---------------- attention: two interleaved head chains ------
            def emit_ag(h):
                # per-head all-gather within the 4-core batch group, issued as
                # soon as this head's ctx^T is written -> overlaps the
                # remaining heads' compute
                nc.gpsimd.collective_compute(
                    "AllGather",
                    mybir.AluOpType.bypass,
                    replica_groups=[[0, 1, 2, 3], [4, 5, 6, 7]],
                    ins=[cc_in[h].ap().opt()],
                    outs=[cc_out[h].ap().opt()],
                )
                for j in range(2):
                    ct = 2 * h + j
                    nc.sync.dma_start(ctxg_sb[:, ct * T:(ct + 1) * T],
                                      cc_out[h][128 * j:128 * (j + 1), :])

            # ---------------- output projection (four stages) -------------
            # stage s consumes AG(s)'s two c-tiles as soon as that gather
            # lands; partials accumulate in SBUF.  Stage 3 adds bias + stores.
            acc_sb = pers.tile([128, NTT * 256], F32, tag="acc_sb",
                               name="acc_sb")

            def emit_proj_stage(s):
                for tt in range(NTT):
                    po = pp.tile([128, 256], F32,
                                 tag=("ctxX" if tt % 2 == 0 else "ctxY"), bufs=1,
                                 name=f"po_{s}_{tt}")
                    for k in range(2):
                        ct = 2 * s + k
                        nc.tensor.matmul(
                            po[:],
                            lhsT=ctxg_sb[:, ct * T + tt * 128:
                                         ct * T + (tt + 1) * 128],
                            rhs=wo_sb[:, ct * 256:(ct + 1) * 256],
                            start=(k == 0), stop=(k == 1))
                    a = acc_sb[:, tt * 256:(tt + 1) * 256]
                    if s == 0:
                        nc.vector.tensor_add(a, po[:], bo_sb[:])
                    elif s < 3:
                        nc.vector.tensor_add(a, po[:], a)
                    else:
                        o_sb = sp.tile([128, 256], F32, tag="o_sb", bufs=3,
                                       name=f"o_{tt}")
                        nc.vector.tensor_add(o_sb[:], po[:], a)
                        nc.sync.dma_start(out[128 * tt:128 * (tt + 1), :],
                                          o_sb[:])

            def emit_attn_pair(hx, hy, qc):
                # heads hx (partition rows 0-63) and hy (rows 64-127) advance
                # in lockstep; their K=64 logits matmuls are emitted adjacent
                # so the PE packs them into disjoint row-groups.
                nkt = 4 * qc + 4
                ctxs = {}
                lgs = {}
                exs = {}
                for grp in range(nkt // 2):
                    for h, cn in ((hx, "X"), (hy, "Y")):
                        if grp == 0:
                            ctxs[cn] = pp.tile([VW, TQ], F32, tag=f"ctx{cn}",
                                               bufs=1, name=f"ctx_{h}_{qc}")
                        lgs[cn] = pp.tile([128, 2 * TQ], F32, tag=f"lg{cn}",
                                          bufs=(2 if cn == "X" else 1),
                                          name=f"lg_{h}_{qc}_{grp}")
                        exs[cn] = sp.tile([128, 2 * TQ], BF16, tag=f"ex{cn}",
                                          bufs=5, name=f"ex_{h}_{qc}_{grp}")
                    for j in range(2):
                        kt = 2 * grp + j
                        for h, cn in ((hx, "X"), (hy, "Y")):
                            p, half = h // 2, h % 2
                            r0 = DK * half
                            nc.tensor.matmul(
                                lgs[cn][:, j * TQ:(j + 1) * TQ],
                                lhsT=kT_sb[r0:r0 + DK,
                                           p * T + kt * 128:p * T + (kt + 1) * 128],
                                rhs=qT_sb[r0:r0 + DK,
                                          p * T + qc * TQ:p * T + (qc + 1) * TQ],
                                start=True, stop=True)
                    for h, cn in ((hx, "X"), (hy, "Y")):
                        nc.scalar.activation(exs[cn][:], lgs[cn][:], AF.Exp,
                                             scale=SCALE)
                    for h, cn in ((hx, "X"), (hy, "Y")):
                        for j in range(2):
                            kt = 2 * grp + j
                            if kt >= 4 * qc:
                                r = kt - 4 * qc
                                nc.vector.tensor_mul(
                                    exs[cn][:, j * TQ:(j + 1) * TQ],
                                    exs[cn][:, j * TQ:(j + 1) * TQ],
                                    mask_sb[:, r * TQ:(r + 1) * TQ])
                            nc.tensor.matmul(
                                ctxs[cn][:],
                                lhsT=v_sb[:, kt * HPC * VW + VW * h:
                                          kt * HPC * VW + VW * h + VW],
                                rhs=exs[cn][:, j * TQ:(j + 1) * TQ],
                                start=(kt == 0), stop=(kt == nkt - 1))
                for h, cn in ((hx, "X"), (hy, "Y")):
                    ctx = ctxs[cn]
                    # drain psum fast (DVE only) so the attention pipeline
                    # never waits on the division chain -- the division below
                    # is SBUF-only, so a Pool queue blocked on an in-flight
                    # collective cannot hold a psum bank hostage.
                    ctxu = sp.tile([DK, TQ], F32, tag=f"ctxu{cn}", bufs=4,
                                   name=f"ctxu_{h}_{qc}")
                    nc.vector.tensor_copy(ctxu[:], ctx[0:DK, :])
                    dn = sp.tile([1, TQ], F32, tag=f"dn{cn}", bufs=4,
                                 name=f"dn_{h}_{qc}")
                    nc.vector.tensor_copy(dn[:], ctx[DK:DK + 1, :])
                    rc = sp.tile([1, TQ], F32, tag=f"rc{cn}", bufs=2,
                                 name=f"rc_{h}_{qc}")
                    nc.vector.reciprocal_approx_fast(rc[:], dn[:])
                    rcb = sp.tile([DK, TQ], F32, tag=f"rcb{cn}", bufs=2,
                                  name=f"rcb_{h}_{qc}")
                    nc.gpsimd.partition_broadcast(rcb[:], rc[:])
                    ctxd = sp.tile([DK, TQ], BF16, tag=f"ctxd{cn}", bufs=2,
                                   name=f"ctxd_{h}_{qc}")
                    nc.vector.tensor_mul(ctxd[:], ctxu[:], rcb[:])
                    nc.sync.dma_start(cc_in[h][:, qc * TQ:(qc + 1) * TQ],
                                      ctxd[:])

            emit_qk(0)
            emit_v()
            emit_qk(1)
            for qc in range(NQC):
                emit_attn_pair(0, 1, qc)
            emit_ag(0)
            emit_ag(1)
            for qc in range(NQC):
                emit_attn_pair(2, 3, qc)
            # AG(0)/AG(1) have landed; their half of the output projection
            # fills PE while heads 2,3 wind down
            emit_proj_stage(0)
            emit_proj_stage(1)
            emit_ag(2)
            emit_ag(3)
            emit_proj_stage(2)
            emit_proj_stage(3)


    nc.compile()
    return nc


def _masks_np():
    jj = np.arange(128)[:, None]
    ii = np.arange(TQ)[None, :]
    m = np.zeros((128, 4 * TQ), np.float32)
    for r in range(4):
        m[:, r * TQ:(r + 1) * TQ] = (jj + 128 * r <= ii)
    return m.astype(ml_dtypes.bfloat16)


def _wo_reorder(Wo, g):
    # device c-tile slot ct=2h+j must hold W_o rows for heads (8j+h, 8j+4+h)
    blocks = []
    for h in range(HPC):
        for j in range(2):
            for r in (2 * j, 2 * j + 1):
                gh = 4 * r + h
                blocks.append(Wo[gh * DK:(gh + 1) * DK, 256 * g:256 * (g + 1)])
    return np.ascontiguousarray(np.concatenate(blocks, axis=0))


def _shard_inputs(x, Wqkv, bqkv, Wo, bo_v):
    bf = ml_dtypes.bfloat16
    masks = _masks_np()
    in_maps = []
    for c in range(NCORES):
        b, g = c // 4, c % 4
        h0 = 4 * g
        q0 = h0 * DK
        wv = np.zeros((D, HPC * VW), np.float32)
        bv = np.zeros((HPC * VW,), np.float32)
        for j in range(HPC):
            wv[:, VW * j:VW * j + DK] = Wqkv[:, 2 * D + (h0 + j) * DK:
                                             2 * D + (h0 + j + 1) * DK]
            bv[VW * j:VW * j + DK] = bqkv[2 * D + (h0 + j) * DK:
                                          2 * D + (h0 + j + 1) * DK]
            bv[VW * j + DK] = 1.0
        in_maps.append({
            "xt": np.ascontiguousarray(x[b].T).astype(bf),
            "wq": np.ascontiguousarray(Wqkv[:, q0:q0 + 256]).astype(bf),
            "wk": np.ascontiguousarray(Wqkv[:, D + q0:D + q0 + 256]).astype(bf),
            "wv": wv.astype(bf),
            "wo": _wo_reorder(Wo, g).astype(bf),
            "bq": np.stack([bqkv[q0:q0 + 128], bqkv[q0 + 128:q0 + 256]],
                           axis=1).astype(np.float32).copy(),
            "bk": np.stack([bqkv[D + q0:D + q0 + 128],
                            bqkv[D + q0 + 128:D + q0 + 256]],
                           axis=1).astype(np.float32).copy(),
            "bv": np.ascontiguousarray(
                np.broadcast_to(bv, (128, HPC * VW))).astype(np.float32),
            "bo": np.ascontiguousarray(
                np.broadcast_to(bo_v[256 * g:256 * (g + 1)], (128, 256))
            ).astype(np.float32),
            "masks": masks,
        })
    return in_maps


def kernel(**inputs):
    x = np.asarray(inputs["x"], np.float32)
    Wqkv = np.asarray(inputs["W_qkv"], np.float32)
    bqkv = np.asarray(inputs["b_qkv"], np.float32)
    Wo = np.asarray(inputs["W_o"], np.float32)
    bo_v = np.asarray(inputs["b_o"], np.float32)

    if "nc" not in _cache:
        _cache["nc"] = _build()
    nc = _cache["nc"]

    in_maps = _shard_inputs(x, Wqkv, bqkv, Wo, bo_v)
    res = bass_utils.run_bass_kernel_spmd(
        nc, in_maps, core_ids=list(range(NCORES)), trace=TRACE)
    LAST_RESULT["exec_time_ns"] = res.exec_time_ns
    LAST_RESULT["res"] = res

    out = np.empty((2, T, D), np.float32)
    for c in range(NCORES):
        out[c // 4, :, 256 * (c % 4):256 * (c % 4 + 1)] = res.results[c]["out"]
    return out



# revision 9
# speedup vs baseline: 1.1766x; 1.1766x over previous
"""Distributed causal multi-head attention for Trainium2 (8 NeuronCores).

Reference computes, for x [2, 2048, 1024]:
    qkv = x @ W_qkv + b_qkv ; split into q,k,v heads (16 heads, d_k=64)
    causal softmax attention per head
    out = ctx @ W_o + b_o

Sharding (data + head parallel): core c handles batch b=c//4 and heads
H = [4g..4g+3] with g=c%4.  Per core:
  - q^T,k^T in [dk, T] layout (head pairs packed into 128 partitions),
    v in [T, dk] layout augmented with a ones column (so the AV matmul
    also produces softmax denominators),
  - causal T x T attention per head pair; exp on the Scalar engine is the
    inner-loop ceiling, so QKV / V / output-projection matmuls are emitted
    as filler between attention groups to keep the PE busy during exp,
  - 8 fine-grained AllGathers (head-pair x 512-wide q-chunk) within the
    4-core batch group, issued as soon as each chunk's ctx^T is drained,
  - output projection accumulates per (pair, qc) chunk in PSUM as the
    gathers land; each core produces a disjoint 256-column slice of out.
Host-side: shard prep (transpose/slice/bf16-cast) and a pure concat of the
8 output column-slices.  All FLOPs (matmuls, softmax, reductions) on device.
"""

import numpy as np
import ml_dtypes

import concourse.bass as bass
import concourse.mybir as mybir
import concourse.tile as tile
from concourse import bacc
from concourse import bass_utils

BF16 = mybir.dt.bfloat16
F32 = mybir.dt.float32
AF = mybir.ActivationFunctionType

T = 2048
D = 1024
NH = 16
HPC = 4  # heads per core
DK = 64
NCORES = 8
TQ = 512  # q-chunk
NQC = T // TQ  # 4
NDT = D // 128  # 8 d-tiles
NTT = T // 128  # 16 t-tiles
VW = DK + 1  # 65: v columns per head incl. ones column
VB = HPC * VW  # 260
SCALE = 1.0 / 8.0  # 1/sqrt(DK)

TRACE = False
LAST_RESULT = {}

_cache = {}


def _build():
    nc = bacc.Bacc("TRN2", target_bir_lowering=False, debug=False,
                   num_devices=NCORES)

    xt = nc.declare_dram_parameter("xt", [D, T], BF16, False)
    wq = nc.declare_dram_parameter("wq", [D, 256], BF16, False)
    wk = nc.declare_dram_parameter("wk", [D, 256], BF16, False)
    wv = nc.declare_dram_parameter("wv", [D, VB], BF16, False)
    wo = nc.declare_dram_parameter("wo", [D, 256], BF16, False)
    bq = nc.declare_dram_parameter("bq", [128, 2], F32, False)
    bk = nc.declare_dram_parameter("bk", [128, 2], F32, False)
    bv = nc.declare_dram_parameter("bv", [128, VB], F32, False)
    bo = nc.declare_dram_parameter("bo", [128, 256], F32, False)
    tri = nc.declare_dram_parameter("tri", [128, 4 * TQ], BF16, False)
    out = nc.declare_dram_parameter("out", [T, 256], F32, True)

    # one AllGather per (head-pair, q-chunk): k = pair*4 + qc
    cc_in = [nc.dram_tensor(f"cc_in{k}", [128, TQ], BF16) for k in range(8)]
    cc_out = [nc.dram_tensor(f"cc_out{k}", [512, TQ], BF16) for k in range(8)]

    with tile.TileContext(nc) as tc, tc.tile_pool(name="pers", bufs=1) as pers:
        xt_sb = pers.tile([128, NDT * T], BF16, tag="xt_sb", name="xt_sb")
        wq_sb = pers.tile([128, NDT * 256], BF16, tag="wq_sb", name="wq_sb")
        wk_sb = pers.tile([128, NDT * 256], BF16, tag="wk_sb", name="wk_sb")
        wv_sb = pers.tile([128, NDT * VB], BF16, tag="wv_sb", name="wv_sb")
        wo_sb = pers.tile([128, NDT * 256], BF16, tag="wo_sb", name="wo_sb")
        bq_sb = pers.tile([128, 2], F32, tag="bq_sb", name="bq_sb")
        bk_sb = pers.tile([128, 2], F32, tag="bk_sb", name="bk_sb")
        bv_sb = pers.tile([128, VB], F32, tag="bv_sb", name="bv_sb")
        bo_sb = pers.tile([128, 256], F32, tag="bo_sb", name="bo_sb")
        tri_sb = pers.tile([128, 4 * TQ], BF16, tag="tri_sb", name="tri_sb")
        qT_sb = pers.tile([128, 2 * T], BF16, tag="qT_sb", name="qT_sb")
        kT_sb = pers.tile([128, 2 * T], BF16, tag="kT_sb", name="kT_sb")
        v_sb = pers.tile([128, NTT * VB], BF16, tag="v_sb", name="v_sb")
        # ctxg slot s = pair*4 + peer j, cols s*T + qc*TQ
        ctxg_sb = pers.tile([128, 8 * T], BF16, tag="ctxg_sb", name="ctxg_sb")
        acc_sb = pers.tile([128, NTT * 256], F32, tag="acc_sb", name="acc_sb")

        # ---- input DMAs in need-order (sync queue is FIFO) ----
        for d in range(NDT):
            nc.sync.dma_start(wq_sb[:, d * 256:(d + 1) * 256],
                              wq[128 * d:128 * (d + 1), :])
            nc.sync.dma_start(wk_sb[:, d * 256:(d + 1) * 256],
                              wk[128 * d:128 * (d + 1), :])
            nc.sync.dma_start(
                xt_sb[:, d * T:d * T + TQ],
                xt[128 * d:128 * (d + 1), 0:TQ])
        nc.sync.dma_start(bq_sb[:], bq[:])
        nc.sync.dma_start(bk_sb[:], bk[:])
        nc.sync.dma_start(tri_sb[:], tri[:])
        for d in range(NDT):
            nc.sync.dma_start(wv_sb[:, d * VB:(d + 1) * VB],
                              wv[128 * d:128 * (d + 1), :])
        nc.sync.dma_start(bv_sb[:], bv[:])
        for qc in range(1, NQC):
            for d in range(NDT):
                nc.sync.dma_start(
                    xt_sb[:, d * T + qc * TQ:d * T + (qc + 1) * TQ],
                    xt[128 * d:128 * (d + 1), qc * TQ:(qc + 1) * TQ])
        for d in range(NDT):
            nc.sync.dma_start(wo_sb[:, d * 256:(d + 1) * 256],
                              wo[128 * d:128 * (d + 1), :])
        nc.sync.dma_start(bo_sb[:], bo[:])

        with (
            tc.tile_pool(name="pp", space="PSUM", bufs=2) as pp,
            tc.tile_pool(name="sp", space="SBUF", bufs=2) as sp,
        ):
            # ---------------- emitters used as PE filler ----------------
            def emit_qk_tile(p, qc, which):
                wsb, bsb, dst = ((wq_sb, bq_sb, qT_sb) if which == "q"
                                 else (wk_sb, bk_sb, kT_sb))
                ps = pp.tile([128, TQ], F32, tag="aux", bufs=2,
                             name=f"ps{which}_{p}_{qc}")
                for d in range(NDT):
                    nc.tensor.matmul(
                        ps[:],
                        lhsT=wsb[:, d * 256 + 128 * p:d * 256 + 128 * p + 128],
                        rhs=xt_sb[:, d * T + qc * TQ:d * T + (qc + 1) * TQ],
                        start=(d == 0), stop=(d == NDT - 1))
                nc.vector.tensor_scalar_add(
                    dst[:, p * T + qc * TQ:p * T + (qc + 1) * TQ],
                    ps[:], bsb[:, p:p + 1])

            def emit_v_tile(tt):
                ps = pp.tile([128, VB], F32, tag="aux", bufs=2,
                             name=f"psv_{tt}")
                for d in range(NDT):
                    nc.tensor.matmul(
                        ps[:],
                        lhsT=xt_sb[:, d * T + tt * 128:d * T + (tt + 1) * 128],
                        rhs=wv_sb[:, d * VB:(d + 1) * VB],
                        start=(d == 0), stop=(d == NDT - 1))
                nc.vector.tensor_add(v_sb[:, tt * VB:(tt + 1) * VB],
                                     ps[:], bv_sb[:])

            def emit_proj(pair, qc, final):
                # consume AG(pair, qc): 4 c-tile loads + 4 tt-proj partials
                k = pair * 4 + qc
                for j in range(4):
                    s = pair * 4 + j
                    nc.sync.dma_start(
                        ctxg_sb[:, s * T + qc * TQ:s * T + (qc + 1) * TQ],
                        cc_out[k][128 * j:128 * (j + 1), :])
                for tt in range(4 * qc, 4 * qc + 4):
                    po = pp.tile([128, 256], F32, tag="aux", bufs=2,
                                 name=f"po_{pair}_{tt}")
                    for j in range(4):
                        s = pair * 4 + j
                        nc.tensor.matmul(
                            po[:],
                            lhsT=ctxg_sb[:, s * T + tt * 128:
                                         s * T + (tt + 1) * 128],
                            rhs=wo_sb[:, s * 256:(s + 1) * 256],
                            start=(j == 0), stop=(j == 3))
                    a = acc_sb[:, tt * 256:(tt + 1) * 256]
                    if not final:
                        nc.vector.tensor_add(a, po[:], bo_sb[:])
                    else:
                        o = sp.tile([128, 256], F32, tag="o", bufs=3,
                                    name=f"o_{tt}")
                        nc.vector.tensor_add(o[:], po[:], a)
                        nc.sync.dma_start(out[tt * 128:(tt + 1) * 128, :],
                                          o[:])

            # ---------------- attention (pair = 2 heads in lockstep) -----
            def emit_attn(pair, qc, filler):
                p = pair
                nkt = 4 * qc + 4
                ctxs = {}
                for g in range(nkt // 2):
                    lgs = {}
                    exs = {}
                    for cn, r0 in (("X", 0), ("Y", 64)):
                        lgs[cn] = pp.tile([128, 2 * TQ], F32, tag=f"lg{cn}",
                                          bufs=1, name=f"lg{cn}_{pair}_{qc}_{g}")
                        if g == 0:
                            ctxs[cn] = pp.tile([VW, TQ], F32, tag=f"ctx{cn}",
                                               bufs=1, name=f"ctx{cn}_{pair}_{qc}")
                    for j in range(2):
                        kt = 2 * g + j
                        for cn, r0 in (("X", 0), ("Y", 64)):
                            nc.tensor.matmul(
                                lgs[cn][:, j * TQ:(j + 1) * TQ],
                                lhsT=kT_sb[r0:r0 + DK,
                                           p * T + kt * 128:p * T + (kt + 1) * 128],
                                rhs=qT_sb[r0:r0 + DK,
                                          p * T + qc * TQ:p * T + (qc + 1) * TQ],
                                start=True, stop=True)
                    for cn in ("X", "Y"):
                        exs[cn] = sp.tile([128, 2 * TQ], BF16, tag=f"ex{cn}",
                                          bufs=3, name=f"ex{cn}_{pair}_{qc}_{g}")
                        nc.scalar.activation(exs[cn][:], lgs[cn][:], AF.Exp,
                                             scale=SCALE)
                    # PE filler while the Scalar engine runs exp
                    filler.step()
                    for half, cn in ((0, "X"), (1, "Y")):
                        h = 2 * pair + half
                        for j in range(2):
                            kt = 2 * g + j
                            r = kt - 4 * qc
                            vcol = kt * VB + VW * h
                            ex = exs[cn]
                            ctx = ctxs[cn]
                            if r >= 0:
                                nc.vector.tensor_mul(
                                    ex[:, j * TQ:(j + 1) * TQ],
                                    ex[:, j * TQ:(j + 1) * TQ],
                                    tri_sb[:, r * TQ:(r + 1) * TQ])
                            nc.tensor.matmul(
                                ctx[:],
                                lhsT=v_sb[:, vcol:vcol + VW],
                                rhs=ex[:, j * TQ:(j + 1) * TQ],
                                start=(kt == 0), stop=(kt == nkt - 1))
                    filler.step()
                # normalize + drain + trigger this chunk's AllGather
                k = pair * 4 + qc
                for half, cn in ((0, "X"), (1, "Y")):
                    ctx = ctxs[cn]
                    dn = sp.tile([1, TQ], F32, tag=f"dn{cn}", bufs=2,
                                 name=f"dn{cn}_{pair}_{qc}")
                    nc.vector.tensor_copy(dn[:], ctx[DK:DK + 1, :])
                    rc = sp.tile([1, TQ], F32, tag=f"rc{cn}", bufs=2,
                                 name=f"rc{cn}_{pair}_{qc}")
                    nc.vector.reciprocal_approx_fast(rc[:], dn[:])
                    rcb = sp.tile([DK, TQ], F32, tag=f"rcb{cn}", bufs=2,
                                  name=f"rcb{cn}_{pair}_{qc}")
                    nc.gpsimd.partition_broadcast(rcb[:], rc[:])
                    ctxd = sp.tile([DK, TQ], BF16, tag=f"ctxd{cn}", bufs=2,
                                   name=f"ctxd{cn}_{pair}_{qc}")
                    nc.vector.tensor_mul(ctxd[:], ctx[0:DK, :], rcb[:])
                    nc.sync.dma_start(cc_in[k][DK * half:DK * (half + 1), :],
                                      ctxd[:])
                nc.gpsimd.collective_compute(
                    "AllGather",
                    mybir.AluOpType.bypass,
                    replica_groups=[[0, 1, 2, 3], [4, 5, 6, 7]],
                    ins=[cc_in[k].ap().opt()],
                    outs=[cc_out[k].ap().opt()],
                )

            # ---------------- filler dispenser ----------------
            class Filler:
                def __init__(self):
                    self.items = []

                def load(self, items):
                    self.items = list(items) + self.items

                def step(self):
                    if self.items:
                        self.items.pop(0)()

                def drain(self):
                    while self.items:
                        self.items.pop(0)()

            filler = Filler()

            def qk_items(p, qc):
                return [lambda p=p, qc=qc: emit_qk_tile(p, qc, "q"),
                        lambda p=p, qc=qc: emit_qk_tile(p, qc, "k")]

            def v_items(lo, hi):
                return [lambda tt=tt: emit_v_tile(tt) for tt in range(lo, hi)]

            def proj_items(pair, qc, final):
                return [lambda: emit_proj(pair, qc, final)]

            # ---------------- schedule ----------------
            emit_qk_tile(0, 0, "q")
            emit_qk_tile(0, 0, "k")
            for tt in range(4):
                emit_v_tile(tt)

            phase_fillers = {
                (0, 0): qk_items(0, 1) + v_items(4, 8),
                (0, 1): qk_items(0, 2) + v_items(8, 12),
                (0, 2): qk_items(0, 3) + v_items(12, 16) + qk_items(1, 0),
                (0, 3): qk_items(1, 1) + proj_items(0, 0, False),
                (1, 0): qk_items(1, 2) + proj_items(0, 1, False),
                (1, 1): qk_items(1, 3) + proj_items(0, 2, False),
                (1, 2): proj_items(0, 3, False),
                (1, 3): proj_items(1, 0, True) + proj_items(1, 1, True),
            }
            for pair in range(2):
                for qc in range(NQC):
                    filler.load(phase_fillers[(pair, qc)])
                    emit_attn(pair, qc, filler)
                    filler.drain()
            emit_proj(1, 2, True)
            emit_proj(1, 3, True)

    nc.compile()
    return nc


def _tri_np():
    jj = np.arange(128)[:, None]
    ii = np.arange(TQ)[None, :]
    m = np.zeros((128, 4 * TQ), np.float32)
    for r in range(4):
        m[:, r * TQ:(r + 1) * TQ] = (jj + 128 * r <= ii)
    return m.astype(ml_dtypes.bfloat16)


def _wo_reorder(Wo, g):
    # ctxg slot s = pair*4 + j holds rows of global heads (4j+2*pair, +1)
    blocks = []
    for pair in range(2):
        for j in range(4):
            for hh in (4 * j + 2 * pair, 4 * j + 2 * pair + 1):
                blocks.append(Wo[hh * DK:(hh + 1) * DK, 256 * g:256 * (g + 1)])
    return np.ascontiguousarray(np.concatenate(blocks, axis=0))


def _shard_inputs(x, Wqkv, bqkv, Wo, bo_v):
    bf = ml_dtypes.bfloat16
    tri = _tri_np()
    in_maps = []
    for c in range(NCORES):
        b, g = c // 4, c % 4
        h0 = 4 * g
        q0 = h0 * DK
        wv = np.zeros((D, VB), np.float32)
        bv = np.zeros((VB,), np.float32)
        for j in range(HPC):
            wv[:, VW * j:VW * j + DK] = Wqkv[:, 2 * D + (h0 + j) * DK:
                                             2 * D + (h0 + j + 1) * DK]
            bv[VW * j:VW * j + DK] = bqkv[2 * D + (h0 + j) * DK:
                                          2 * D + (h0 + j + 1) * DK]
            bv[VW * j + DK] = 1.0
        in_maps.append({
            "xt": np.ascontiguousarray(x[b].T).astype(bf),
            "wq": np.ascontiguousarray(Wqkv[:, q0:q0 + 256]).astype(bf),
            "wk": np.ascontiguousarray(Wqkv[:, D + q0:D + q0 + 256]).astype(bf),
            "wv": wv.astype(bf),
            "wo": _wo_reorder(Wo, g).astype(bf),
            "bq": np.stack([bqkv[q0:q0 + 128], bqkv[q0 + 128:q0 + 256]],
                           axis=1).astype(np.float32).copy(),
            "bk": np.stack([bqkv[D + q0:D + q0 + 128],
                            bqkv[D + q0 + 128:D + q0 + 256]],
                           axis=1).astype(np.float32).copy(),
            "bv": np.ascontiguousarray(
                np.broadcast_to(bv, (128, VB))).astype(np.float32),
            "bo": np.ascontiguousarray(
                np.broadcast_to(bo_v[256 * g:256 * (g + 1)], (128, 256))
            ).astype(np.float32),
            "tri": tri,
        })
    return in_maps


def kernel(**inputs):
    x = np.asarray(inputs["x"], np.float32)
    Wqkv = np.asarray(inputs["W_qkv"], np.float32)
    bqkv = np.asarray(inputs["b_qkv"], np.float32)
    Wo = np.asarray(inputs["W_o"], np.float32)
    bo_v = np.asarray(inputs["b_o"], np.float32)

    if "nc" not in _cache:
        _cache["nc"] = _build()
    nc = _cache["nc"]

    in_maps = _shard_inputs(x, Wqkv, bqkv, Wo, bo_v)
    res = bass_utils.run_bass_kernel_spmd(
        nc, in_maps, core_ids=list(range(NCORES)), trace=TRACE)
    LAST_RESULT["exec_time_ns"] = res.exec_time_ns
    LAST_RESULT["res"] = res

    out = np.empty((2, T, D), np.float32)
    for c in range(NCORES):
        out[c // 4, :, 256 * (c % 4):256 * (c % 4 + 1)] = res.results[c]["out"]
    return out


# revision 13
# speedup vs baseline: 1.2085x; 1.0271x over previous
"""Distributed causal multi-head attention for Trainium2 (8 NeuronCores).

Reference computes, for x [2, 2048, 1024]:
    qkv = x @ W_qkv + b_qkv ; split into q,k,v heads (16 heads, d_k=64)
    causal softmax attention per head
    out = ctx @ W_o + b_o

Sharding (data + head parallel): core c handles batch b=c//4 and heads
H = [4g..4g+3] with g=c%4.  Per core:
  - q^T,k^T in [dk, T] layout (head pairs packed into 128 partitions),
    v in [T, dk] layout augmented with a ones column (so the AV matmul
    also produces softmax denominators),
  - causal T x T attention per head pair; exp on the Scalar engine is the
    inner-loop ceiling, so QKV / V / output-projection matmuls are emitted
    as filler between attention groups to keep the PE busy during exp,
  - 8 fine-grained AllGathers (head-pair x 512-wide q-chunk) within the
    4-core batch group, issued as soon as each chunk's ctx^T is drained,
  - output projection accumulates per (pair, qc) chunk in PSUM as the
    gathers land; each core produces a disjoint 256-column slice of out.
Host-side: shard prep (transpose/slice/bf16-cast) and a pure concat of the
8 output column-slices.  All FLOPs (matmuls, softmax, reductions) on device.
"""

import numpy as np
import ml_dtypes

import concourse.bass as bass
import concourse.mybir as mybir
import concourse.tile as tile
from concourse import bacc
from concourse import bass_utils

BF16 = mybir.dt.bfloat16
F32 = mybir.dt.float32
AF = mybir.ActivationFunctionType

T = 2048
D = 1024
NH = 16
HPC = 4  # heads per core
DK = 64
NCORES = 8
TQ = 512  # q-chunk
NQC = T // TQ  # 4
NDT = D // 128  # 8 d-tiles
NTT = T // 128  # 16 t-tiles
VW = DK + 1  # 65: v columns per head incl. ones column
VB = HPC * VW  # 260
SCALE = 1.0 / 8.0  # 1/sqrt(DK)

TRACE = False
LAST_RESULT = {}

_cache = {}


def _build():
    nc = bacc.Bacc("TRN2", target_bir_lowering=False, debug=False,
                   num_devices=NCORES)

    xt = nc.declare_dram_parameter("xt", [D, T], BF16, False)
    wq = nc.declare_dram_parameter("wq", [D, 256], BF16, False)
    wk = nc.declare_dram_parameter("wk", [D, 256], BF16, False)
    wv = nc.declare_dram_parameter("wv", [D, VB], BF16, False)
    wo = nc.declare_dram_parameter("wo", [D, 256], BF16, False)
    bq = nc.declare_dram_parameter("bq", [128, 2], F32, False)
    bk = nc.declare_dram_parameter("bk", [128, 2], F32, False)
    bv = nc.declare_dram_parameter("bv", [128, VB], F32, False)
    bo = nc.declare_dram_parameter("bo", [128, 256], F32, False)
    tri = nc.declare_dram_parameter("tri", [128, 4 * TQ], BF16, False)
    out = nc.declare_dram_parameter("out", [T, 256], F32, True)

    # one AllGather per (head-pair, q-chunk): k = pair*4 + qc
    cc_in = [nc.dram_tensor(f"cc_in{k}", [128, TQ], BF16) for k in range(8)]
    cc_out = [nc.dram_tensor(f"cc_out{k}", [512, TQ], BF16) for k in range(8)]
    # tiny warmup collective: absorbs the one-time CC-stream setup (~15us)
    cc_w_in = nc.dram_tensor("cc_w_in", [4, 4], BF16)
    cc_w_out = nc.dram_tensor("cc_w_out", [16, 4], BF16)

    with tile.TileContext(nc) as tc, tc.tile_pool(name="pers", bufs=1) as pers:
        xt_sb = pers.tile([128, NDT * T], BF16, tag="xt_sb", name="xt_sb")
        wq_sb = pers.tile([128, NDT * 256], BF16, tag="wq_sb", name="wq_sb")
        wk_sb = pers.tile([128, NDT * 256], BF16, tag="wk_sb", name="wk_sb")
        wv_sb = pers.tile([128, NDT * VB], BF16, tag="wv_sb", name="wv_sb")
        wo_sb = pers.tile([128, NDT * 256], BF16, tag="wo_sb", name="wo_sb")
        bq_sb = pers.tile([128, 2], F32, tag="bq_sb", name="bq_sb")
        bk_sb = pers.tile([128, 2], F32, tag="bk_sb", name="bk_sb")
        bv_sb = pers.tile([128, VB], F32, tag="bv_sb", name="bv_sb")
        bo_sb = pers.tile([128, 256], F32, tag="bo_sb", name="bo_sb")
        tri_sb = pers.tile([128, 4 * TQ], BF16, tag="tri_sb", name="tri_sb")
        qT_sb = pers.tile([128, 2 * T], BF16, tag="qT_sb", name="qT_sb")
        kT_sb = pers.tile([128, 2 * T], BF16, tag="kT_sb", name="kT_sb")
        v_sb = pers.tile([128, NTT * VB], BF16, tag="v_sb", name="v_sb")
        # ctxg slot s = pair*4 + peer j, cols s*T + qc*TQ
        ctxg_sb = pers.tile([128, 8 * T], BF16, tag="ctxg_sb", name="ctxg_sb")
        acc_sb = pers.tile([128, NTT * 256], F32, tag="acc_sb", name="acc_sb")

        # ---- warmup collective: no data deps, fires immediately ----
        nc.gpsimd.collective_compute(
            "AllGather",
            mybir.AluOpType.bypass,
            replica_groups=[[0, 1, 2, 3], [4, 5, 6, 7]],
            ins=[cc_w_in.ap().opt()],
            outs=[cc_w_out.ap().opt()],
        )

        # ---- input DMAs split over the two HWDGE queues ----
        # scalar queue: weights for Q/K (done before the first exp needs it)
        for d in range(NDT):
            nc.scalar.dma_start(wq_sb[:, d * 256:(d + 1) * 256],
                                wq[128 * d:128 * (d + 1), :])
            nc.scalar.dma_start(wk_sb[:, d * 256:(d + 1) * 256],
                                wk[128 * d:128 * (d + 1), :])
        nc.scalar.dma_start(bq_sb[:], bq[:])
        nc.scalar.dma_start(bk_sb[:], bk[:])
        nc.scalar.dma_start(tri_sb[:], tri[:])
        # sync queue: x tiles + V weights; later x chunks emitted as filler
        for d in range(NDT):
            nc.sync.dma_start(
                xt_sb[:, d * T:d * T + TQ],
                xt[128 * d:128 * (d + 1), 0:TQ])
        for d in range(NDT):
            nc.sync.dma_start(wv_sb[:, d * VB:(d + 1) * VB],
                              wv[128 * d:128 * (d + 1), :])
        nc.sync.dma_start(bv_sb[:], bv[:])
        for d in range(NDT):
            nc.sync.dma_start(
                xt_sb[:, d * T + TQ:d * T + 2 * TQ],
                xt[128 * d:128 * (d + 1), TQ:2 * TQ])

        def emit_xt_load(qc):
            for d in range(NDT):
                nc.sync.dma_start(
                    xt_sb[:, d * T + qc * TQ:d * T + (qc + 1) * TQ],
                    xt[128 * d:128 * (d + 1), qc * TQ:(qc + 1) * TQ])

        def emit_wo_load():
            for d in range(NDT):
                nc.sync.dma_start(wo_sb[:, d * 256:(d + 1) * 256],
                                  wo[128 * d:128 * (d + 1), :])
            nc.sync.dma_start(bo_sb[:], bo[:])

        with (
            tc.tile_pool(name="pp", space="PSUM", bufs=2) as pp,
            tc.tile_pool(name="sp", space="SBUF", bufs=2) as sp,
        ):
            # ---------------- emitters used as PE filler ----------------
            def emit_qk_tile(p, qc, which):
                wsb, bsb, dst = ((wq_sb, bq_sb, qT_sb) if which == "q"
                                 else (wk_sb, bk_sb, kT_sb))
                ps = pp.tile([128, TQ], F32, tag="aux", bufs=2,
                             name=f"ps{which}_{p}_{qc}")
                for d in range(NDT):
                    nc.tensor.matmul(
                        ps[:],
                        lhsT=wsb[:, d * 256 + 128 * p:d * 256 + 128 * p + 128],
                        rhs=xt_sb[:, d * T + qc * TQ:d * T + (qc + 1) * TQ],
                        start=(d == 0), stop=(d == NDT - 1))
                nc.vector.tensor_scalar_add(
                    dst[:, p * T + qc * TQ:p * T + (qc + 1) * TQ],
                    ps[:], bsb[:, p:p + 1])

            def emit_v_tile(tt):
                ps = pp.tile([128, VB], F32, tag="aux", bufs=2,
                             name=f"psv_{tt}")
                for d in range(NDT):
                    nc.tensor.matmul(
                        ps[:],
                        lhsT=xt_sb[:, d * T + tt * 128:d * T + (tt + 1) * 128],
                        rhs=wv_sb[:, d * VB:(d + 1) * VB],
                        start=(d == 0), stop=(d == NDT - 1))
                nc.vector.tensor_add(v_sb[:, tt * VB:(tt + 1) * VB],
                                     ps[:], bv_sb[:])

            def emit_proj(pair, qc, final):
                # consume AG(pair, qc): 4 c-tile loads + 4 tt-proj partials
                k = pair * 4 + qc
                for j in range(4):
                    s = pair * 4 + j
                    nc.sync.dma_start(
                        ctxg_sb[:, s * T + qc * TQ:s * T + (qc + 1) * TQ],
                        cc_out[k][128 * j:128 * (j + 1), :])
                for tt in range(4 * qc, 4 * qc + 4):
                    po = pp.tile([128, 256], F32, tag="aux", bufs=2,
                                 name=f"po_{pair}_{tt}")
                    for j in range(4):
                        s = pair * 4 + j
                        nc.tensor.matmul(
                            po[:],
                            lhsT=ctxg_sb[:, s * T + tt * 128:
                                         s * T + (tt + 1) * 128],
                            rhs=wo_sb[:, s * 256:(s + 1) * 256],
                            start=(j == 0), stop=(j == 3))
                    a = acc_sb[:, tt * 256:(tt + 1) * 256]
                    if not final:
                        nc.vector.tensor_add(a, po[:], bo_sb[:])
                    else:
                        o = sp.tile([128, 256], F32, tag="o", bufs=3,
                                    name=f"o_{tt}")
                        nc.vector.tensor_add(o[:], po[:], a)
                        nc.sync.dma_start(out[tt * 128:(tt + 1) * 128, :],
                                          o[:])

            # ---------------- attention (pair = 2 heads in lockstep) -----
            def emit_attn(pair, qc, filler):
                p = pair
                nkt = 4 * qc + 4
                ctxs = {}
                for g in range(nkt // 2):
                    lgs = {}
                    exs = {}
                    for cn, r0 in (("X", 0), ("Y", 64)):
                        lgs[cn] = pp.tile([128, 2 * TQ], F32, tag=f"lg{cn}",
                                          bufs=1, name=f"lg{cn}_{pair}_{qc}_{g}")
                        if g == 0:
                            ctxs[cn] = pp.tile([VW, TQ], F32, tag=f"ctx{cn}",
                                               bufs=1, name=f"ctx{cn}_{pair}_{qc}")
                    for j in range(2):
                        kt = 2 * g + j
                        for cn, r0 in (("X", 0), ("Y", 64)):
                            nc.tensor.matmul(
                                lgs[cn][:, j * TQ:(j + 1) * TQ],
                                lhsT=kT_sb[r0:r0 + DK,
                                           p * T + kt * 128:p * T + (kt + 1) * 128],
                                rhs=qT_sb[r0:r0 + DK,
                                          p * T + qc * TQ:p * T + (qc + 1) * TQ],
                                start=True, stop=True)
                    for cn in ("X", "Y"):
                        exs[cn] = sp.tile([128, 2 * TQ], BF16, tag=f"ex{cn}",
                                          bufs=3, name=f"ex{cn}_{pair}_{qc}_{g}")
                        nc.scalar.activation(exs[cn][:], lgs[cn][:], AF.Exp,
                                             scale=SCALE)
                    # PE filler while the Scalar engine runs exp
                    filler.step()
                    for half, cn in ((0, "X"), (1, "Y")):
                        h = 2 * pair + half
                        for j in range(2):
                            kt = 2 * g + j
                            r = kt - 4 * qc
                            vcol = kt * VB + VW * h
                            ex = exs[cn]
                            ctx = ctxs[cn]
                            if r >= 0:
                                nc.vector.tensor_mul(
                                    ex[:, j * TQ:(j + 1) * TQ],
                                    ex[:, j * TQ:(j + 1) * TQ],
                                    tri_sb[:, r * TQ:(r + 1) * TQ])
                            nc.tensor.matmul(
                                ctx[:],
                                lhsT=v_sb[:, vcol:vcol + VW],
                                rhs=ex[:, j * TQ:(j + 1) * TQ],
                                start=(kt == 0), stop=(kt == nkt - 1))
                    filler.step()
                # normalize + drain + trigger this chunk's AllGather
                k = pair * 4 + qc
                for half, cn in ((0, "X"), (1, "Y")):
                    ctx = ctxs[cn]
                    dn = sp.tile([1, TQ], F32, tag=f"dn{cn}", bufs=2,
                                 name=f"dn{cn}_{pair}_{qc}")
                    nc.vector.tensor_copy(dn[:], ctx[DK:DK + 1, :])
                    rc = sp.tile([1, TQ], F32, tag=f"rc{cn}", bufs=2,
                                 name=f"rc{cn}_{pair}_{qc}")
                    nc.vector.reciprocal_approx_fast(rc[:], dn[:])
                    rcb = sp.tile([DK, TQ], F32, tag=f"rcb{cn}", bufs=2,
                                  name=f"rcb{cn}_{pair}_{qc}")
                    nc.gpsimd.partition_broadcast(rcb[:], rc[:])
                    ctxd = sp.tile([DK, TQ], BF16, tag=f"ctxd{cn}", bufs=2,
                                   name=f"ctxd{cn}_{pair}_{qc}")
                    nc.vector.tensor_mul(ctxd[:], ctx[0:DK, :], rcb[:])
                    nc.sync.dma_start(cc_in[k][DK * half:DK * (half + 1), :],
                                      ctxd[:])
                nc.gpsimd.collective_compute(
                    "AllGather",
                    mybir.AluOpType.bypass,
                    replica_groups=[[0, 1, 2, 3], [4, 5, 6, 7]],
                    ins=[cc_in[k].ap().opt()],
                    outs=[cc_out[k].ap().opt()],
                )

            # ---------------- filler dispenser ----------------
            class Filler:
                def __init__(self):
                    self.items = []

                def load(self, items):
                    self.items = list(items) + self.items

                def step(self):
                    if self.items:
                        it = self.items.pop(0)
                        if it is not None:
                            it()

                def drain(self):
                    while self.items:
                        it = self.items.pop(0)
                        if it is not None:
                            it()

            filler = Filler()

            def qk_items(p, qc):
                return [lambda p=p, qc=qc: emit_qk_tile(p, qc, "q"),
                        lambda p=p, qc=qc: emit_qk_tile(p, qc, "k")]

            def v_items(lo, hi):
                return [lambda tt=tt: emit_v_tile(tt) for tt in range(lo, hi)]

            def proj_items(pair, qc, final):
                return [lambda: emit_proj(pair, qc, final)]

            # ---------------- schedule ----------------
            emit_qk_tile(0, 0, "q")
            emit_qk_tile(0, 0, "k")
            for tt in range(4):
                emit_v_tile(tt)

            phase_fillers = {
                (0, 0): [lambda: emit_xt_load(2)] + qk_items(0, 1) + v_items(4, 8),
                (0, 1): [lambda: emit_xt_load(3)] + qk_items(0, 2) + v_items(8, 12),
                (0, 2): [emit_wo_load] + qk_items(0, 3) + v_items(12, 16)
                        + qk_items(1, 0),
                (0, 3): qk_items(1, 1) + proj_items(0, 0, False),
                (1, 0): qk_items(1, 2) + proj_items(0, 1, False),
                (1, 1): qk_items(1, 3) + proj_items(0, 2, False),
                (1, 2): proj_items(0, 3, False),
                (1, 3): proj_items(1, 0, True) + [None, None, None]
                        + proj_items(1, 1, True) + [None] * 8
                        + proj_items(1, 2, True),
            }
            for pair in range(2):
                for qc in range(NQC):
                    filler.load(phase_fillers[(pair, qc)])
                    emit_attn(pair, qc, filler)
                    filler.drain()
            emit_proj(1, 3, True)

    nc.compile()
    return nc


def _tri_np():
    jj = np.arange(128)[:, None]
    ii = np.arange(TQ)[None, :]
    m = np.zeros((128, 4 * TQ), np.float32)
    for r in range(4):
        m[:, r * TQ:(r + 1) * TQ] = (jj + 128 * r <= ii)
    return m.astype(ml_dtypes.bfloat16)


def _wo_reorder(Wo, g):
    # ctxg slot s = pair*4 + j holds rows of global heads (4j+2*pair, +1)
    blocks = []
    for pair in range(2):
        for j in range(4):
            for hh in (4 * j + 2 * pair, 4 * j + 2 * pair + 1):
                blocks.append(Wo[hh * DK:(hh + 1) * DK, 256 * g:256 * (g + 1)])
    return np.ascontiguousarray(np.concatenate(blocks, axis=0))


def _shard_inputs(x, Wqkv, bqkv, Wo, bo_v):
    bf = ml_dtypes.bfloat16
    tri = _tri_np()
    in_maps = []
    for c in range(NCORES):
        b, g = c // 4, c % 4
        h0 = 4 * g
        q0 = h0 * DK
        wv = np.zeros((D, VB), np.float32)
        bv = np.zeros((VB,), np.float32)
        for j in range(HPC):
            wv[:, VW * j:VW * j + DK] = Wqkv[:, 2 * D + (h0 + j) * DK:
                                             2 * D + (h0 + j + 1) * DK]
            bv[VW * j:VW * j + DK] = bqkv[2 * D + (h0 + j) * DK:
                                          2 * D + (h0 + j + 1) * DK]
            bv[VW * j + DK] = 1.0
        in_maps.append({
            "xt": np.ascontiguousarray(x[b].T).astype(bf),
            "wq": np.ascontiguousarray(Wqkv[:, q0:q0 + 256]).astype(bf),
            "wk": np.ascontiguousarray(Wqkv[:, D + q0:D + q0 + 256]).astype(bf),
            "wv": wv.astype(bf),
            "wo": _wo_reorder(Wo, g).astype(bf),
            "bq": np.stack([bqkv[q0:q0 + 128], bqkv[q0 + 128:q0 + 256]],
                           axis=1).astype(np.float32).copy(),
            "bk": np.stack([bqkv[D + q0:D + q0 + 128],
                            bqkv[D + q0 + 128:D + q0 + 256]],
                           axis=1).astype(np.float32).copy(),
            "bv": np.ascontiguousarray(
                np.broadcast_to(bv, (128, VB))).astype(np.float32),
            "bo": np.ascontiguousarray(
                np.broadcast_to(bo_v[256 * g:256 * (g + 1)], (128, 256))
            ).astype(np.float32),
            "tri": tri,
        })
    return in_maps


def kernel(**inputs):
    x = np.asarray(inputs["x"], np.float32)
    Wqkv = np.asarray(inputs["W_qkv"], np.float32)
    bqkv = np.asarray(inputs["b_qkv"], np.float32)
    Wo = np.asarray(inputs["W_o"], np.float32)
    bo_v = np.asarray(inputs["b_o"], np.float32)

    if "nc" not in _cache:
        _cache["nc"] = _build()
    nc = _cache["nc"]

    in_maps = _shard_inputs(x, Wqkv, bqkv, Wo, bo_v)
    res = bass_utils.run_bass_kernel_spmd(
        nc, in_maps, core_ids=list(range(NCORES)), trace=TRACE)
    LAST_RESULT["exec_time_ns"] = res.exec_time_ns
    LAST_RESULT["res"] = res

    out = np.empty((2, T, D), np.float32)
    for c in range(NCORES):
        out[c // 4, :, 256 * (c % 4):256 * (c % 4 + 1)] = res.results[c]["out"]
    return out
